# revision 3
# baseline (speedup 1.0000x reference)
"""CrossAttentionQuerySelector TRN2 kernel v2 (8-core data parallel).

All-bf16 matmul design. Per core (2048 samples, K=7, D=512, H=8, S=3):

Phase A (per 64-sample tile, 4 groups of 16 samples = 112 (n,k)-partitions):
  - kvT [512, 14336] bf16 loaded [128,4,448]; scores computed DIRECTLY in
    (n,k)-partition layout: sc[(n,k),(h,s)] = kvT_g.T @ QsT  (Wk folded into
    the 24 slot queries on host).
  - softmax over k (on partitions): exp on ACT; Z via block-ones matmul
    (ones_bd.T @ exp broadcasts group sums back to all 112 partitions);
    reciprocal + multiply on DVE (bf16 2x).
  - attn diagonalized into a block-diagonal [112, 4*8*48] operand with ONE
    gpsimd local_scatter (per-partition indices, auto-zeroed dest).
  - vh' [(n,k), e] = kvT_g.T @ WvT (4 accumulating mms per group).
  - combine ON PE: per (group, e-chunk) two 48-row matmuls
    oT[e, (s,n)] = vh'.T @ bd_head, written at psum partition offsets 0/64.
Phase B (per slot, 512-sample block, pipelined one supertile behind A):
  - out-proj (+xbr bias via ones-matmul), LN1 stats on DVE, rstd=exp(-ln/2)
    on ACT (shared table set with softmax exp), normalize on DVE,
    q_se = t + (b1n+slot_se) on DVE (bf16 2x), PE transposes -> qT,
  - FFN: ff1+gelu; ff2 accumulates on top of an identity-matmul of q_se
    (residual folded into psum), LN2, store.
Emission order per supertile: B.chunks(st-1), A(st), B.ff1(st-1),
B.ff2(st-1) -- keeps ACT table reloads to 2/supertile and overlaps the
DVE/ACT-heavy A phase with the PE-heavy FFN.
"""

import os
import sys

for _p in ("/opt/trn_rl_repo", "/root/.axon_site/_ro/trn_rl_repo"):
    if os.path.isdir(_p) and _p not in sys.path:
        sys.path.insert(0, _p)

import numpy as np
from contextlib import ExitStack

import concourse.bass as bass
import concourse.tile as tile
from concourse import mybir, bacc
from concourse.bass_utils import run_bass_kernel_spmd

# Steer Exp/Ln onto the combined `natural_log_exp_and_others` table set so
# softmax-exp and the LN rstd (exp(-ln/2)) share one set: hide Exp/Ln in
# every other set handed to the load-insertion pass.
_orig_get_tables = bacc.get_activation_tables


def _patched_get_tables(arch):
    tabs = _orig_get_tables(arch)
    out = {}
    for name, funcs in tabs.items():
        if name != "natural_log_exp_and_others":
            funcs = funcs - {mybir.ActivationFunctionType.Exp,
                             mybir.ActivationFunctionType.Ln}
        out[name] = funcs
    return out


bacc.get_activation_tables = _patched_get_tables

F32 = mybir.dt.float32
BF16 = mybir.dt.bfloat16
I16 = mybir.dt.int16
AX = mybir.AxisListType
ALU = mybir.AluOpType
ACT_F = mybir.ActivationFunctionType

D = 512
H = 8
HD = 64
S = 3
K = 7
B = 8
T = 2048
NCORES = 8
NSAMP = T
ROWS = NSAMP * K               # 14336
ST_SAMP = 512
N_ST = NSAMP // ST_SAMP        # 4
A_SAMP = 64                    # samples per phase-A tile
A_ROWS = A_SAMP * K            # 448
N_A = ST_SAMP // A_SAMP        # 8
G = 16                         # samples per combine group
GP = G * K                     # 112 partitions per group
NG = A_SAMP // G               # 4 groups per A-tile
NC4 = ST_SAMP // 128           # 4 sample chunks per phase-B block
EPS = 1e-5

_CACHE = {}


def _build(has_g1, has_g2, has_b2n, reps=1, comb_bf16=True):
    nc = bacc.Bacc("TRN2", target_bir_lowering=False, debug=False,
                   num_devices=NCORES)

    kvT_d = nc.dram_tensor("kvT", [D, ROWS], BF16, kind="ExternalInput")
    qsT_d = nc.dram_tensor("qsT", [D, 24], BF16, kind="ExternalInput")
    wvT_d = nc.dram_tensor("wvT", [D, D], BF16, kind="ExternalInput")
    owT_d = nc.dram_tensor("owT", [D, D], BF16, kind="ExternalInput")
    w1T_d = nc.dram_tensor("w1T", [D, 2 * D], BF16, kind="ExternalInput")
    w2T_d = nc.dram_tensor("w2T", [2 * D, D], BF16, kind="ExternalInput")
    xbr_d = nc.dram_tensor("xbr", [1, S * D], BF16, kind="ExternalInput")
    ser_d = nc.dram_tensor("ser", [1, S * D], BF16, kind="ExternalInput")
    one_d = nc.dram_tensor("one", [1, 128], BF16, kind="ExternalInput")
    idx_d = nc.dram_tensor("idx", [GP, 96], I16, kind="ExternalInput")
    obd_d = nc.dram_tensor("obd", [GP, GP], BF16, kind="ExternalInput")
    idn_d = nc.dram_tensor("idn", [128, 128], BF16, kind="ExternalInput")
    g1_d = g2_d = b2n_d = None
    if has_g1:
        g1_d = nc.dram_tensor("g1v", [D], F32, kind="ExternalInput")
    if has_g2:
        g2_d = nc.dram_tensor("g2v", [D], F32, kind="ExternalInput")
    if has_b2n:
        b2n_d = nc.dram_tensor("b2nv", [D], F32, kind="ExternalInput")
    out_d = nc.dram_tensor("out", [NSAMP, S, D], F32, kind="ExternalOutput")

    with tile.TileContext(nc) as tc, ExitStack() as ctx:
        cp = ctx.enter_context(tc.tile_pool(name="consts", bufs=1))
        kvp = ctx.enter_context(tc.tile_pool(name="kvp", bufs=3))
        ap_ = ctx.enter_context(tc.tile_pool(name="aphase", bufs=2))
        bdp = ctx.enter_context(tc.tile_pool(name="bdp", bufs=2))
        vsp = ctx.enter_context(tc.tile_pool(name="vsp", bufs=2))
        otp = ctx.enter_context(tc.tile_pool(name="otp", bufs=2))
        tp = ctx.enter_context(tc.tile_pool(name="tp", bufs=2))
        qtp = ctx.enter_context(tc.tile_pool(name="qtp", bufs=2))
        ff1p = ctx.enter_context(tc.tile_pool(name="ff1p", bufs=2))
        yp = ctx.enter_context(tc.tile_pool(name="yp", bufs=2))
        sp = ctx.enter_context(tc.tile_pool(name="smalls", bufs=6))
        pp = ctx.enter_context(tc.tile_pool(name="psum", bufs=2, space="PSUM"))

        # ---- constants ----
        qsT = cp.tile([128, 4, 24], BF16, name="qsT")
        wvT = cp.tile([128, 4, D], BF16, name="wvT")
        owT = cp.tile([128, 4, D], BF16, name="owT")
        w1T = cp.tile([128, 4, 2 * D], BF16, name="w1T")
        w2T = cp.tile([128, 8, D], BF16, name="w2T")
        for k in range(4):
            nc.scalar.dma_start(out=qsT[:, k, :], in_=qsT_d[128 * k:128 * (k + 1), :])
            nc.scalar.dma_start(out=wvT[:, k, :], in_=wvT_d[128 * k:128 * (k + 1), :])
            nc.scalar.dma_start(out=owT[:, k, :], in_=owT_d[128 * k:128 * (k + 1), :])
            nc.scalar.dma_start(out=w1T[:, k, :], in_=w1T_d[128 * k:128 * (k + 1), :])
        for k in range(8):
            nc.scalar.dma_start(out=w2T[:, k, :], in_=w2T_d[128 * k:128 * (k + 1), :])
        xbr = cp.tile([1, S, D], BF16, name="xbr")
        nc.scalar.dma_start(out=xbr, in_=xbr_d.ap().rearrange("p (s n) -> p s n", s=S))
        ser = cp.tile([128, S, D], BF16, name="ser")
        nc.scalar.dma_start(out=ser, in_=bass.AP(
            tensor=ser_d, offset=0,
            ap=[[0, 128], [D, S], [1, D]]))
        ones1 = cp.tile([1, 128], BF16, name="ones1")
        nc.scalar.dma_start(out=ones1, in_=one_d[:])
        idx = cp.tile([GP, 96], I16, name="idx")
        nc.scalar.dma_start(out=idx, in_=idx_d[:, :])
        obd = cp.tile([GP, GP], BF16, name="obd")
        nc.scalar.dma_start(out=obd, in_=obd_d[:, :])
        idn = cp.tile([128, 128], BF16, name="idn")
        nc.scalar.dma_start(out=idn, in_=idn_d[:, :])
        epsb = cp.tile([128, 1], F32, name="epsb")
        nc.vector.memset(epsb, EPS)
        g1b = g2b = b2nb = None
        if has_g1:
            g1b = cp.tile([128, D], F32, name="g1b")
            nc.scalar.dma_start(out=g1b, in_=bass.AP(
                tensor=g1_d, offset=0, ap=[[0, 128], [1, D]]))
        if has_g2:
            g2b = cp.tile([128, D], F32, name="g2b")
            nc.scalar.dma_start(out=g2b, in_=bass.AP(
                tensor=g2_d, offset=0, ap=[[0, 128], [1, D]]))
        if has_b2n:
            b2nb = cp.tile([128, D], F32, name="b2nb")
            nc.scalar.dma_start(out=b2nb, in_=bass.AP(
                tensor=b2n_d, offset=0, ap=[[0, 128], [1, D]]))

        def ln_rstd(x_ap, tag):
            """LN stats of x_ap [128,512] -> (mv [128,2], rstd [128,1])."""
            st6 = sp.tile([128, 6], F32, name=f"st6_{tag}", tag="st6")
            nc.vector.bn_stats(out=st6, in_=x_ap)
            mv = sp.tile([128, 2], F32, name=f"mv_{tag}", tag="mv")
            nc.vector.bn_aggr(out=mv, in_=st6)
            lnv = sp.tile([128, 1], F32, name=f"lnv_{tag}", tag="lnv")
            nc.scalar.activation(lnv, mv[:, 1:2], ACT_F.Ln, bias=epsb[:, 0:1])
            rstd = sp.tile([128, 1], F32, name=f"rstd_{tag}", tag="rstd")
            nc.scalar.activation(rstd, lnv, ACT_F.Exp, scale=-0.5)
            return mv, rstd

        # ---------------- emission helpers ----------------
        def emit_A(st, a):
            """64 samples: scores/softmax/scatter/vh/combine -> oT."""
            oT = oT_tiles[st % 2]
            r0 = (st * ST_SAMP + a * A_SAMP) * K
            kv = kvp.tile([128, 4, A_ROWS], BF16, name=f"kv_{st}_{a}", tag="kv")
            nc.sync.dma_start(
                out=kv,
                in_=bass.AP(tensor=kvT_d, offset=r0,
                            ap=[[ROWS, 128], [128 * ROWS, 4], [1, A_ROWS]]))
            scz = pp.tile([GP, 2 * NG, 24], F32, name=f"scz_{st}_{a}",
                          tag="scz")
            vh_sbs = []
            for g in range(NG):
                vh_ps = pp.tile([128, D], F32, name=f"vh_{st}_{a}_{g}",
                                tag="pbig2")
                for k in range(4):
                    lhsT = kv[:, k, GP * g:GP * (g + 1)]
                    nc.tensor.matmul(scz[:, g, :], lhsT, qsT[:, k, :],
                                     start=(k == 0), stop=(k == 3),
                                     skip_group_check=True)
                    nc.tensor.matmul(vh_ps[0:GP, :], lhsT, wvT[:, k, :],
                                     start=(k == 0), stop=(k == 3),
                                     skip_group_check=True)
                vh_sb = vsp.tile([128, D], BF16, name=f"vs_{st}_{a}_{g}",
                                 tag="vs", bufs=4)
                vh_sbs.append(vh_sb)
                if g % 2 == 0:
                    nc.scalar.copy(vh_sb[0:GP, :], vh_ps[0:GP, :])
                else:
                    nc.vector.tensor_scalar_add(vh_sb[0:GP, :], vh_ps[0:GP, :], 0.0)
            attn_e = ap_.tile([GP, NG, 24], BF16, name=f"ae_{st}_{a}", tag="ae")
            nc.scalar.activation(attn_e, scz[:, 0:NG, :], ACT_F.Exp)
            nc.tensor.matmul(scz[:, NG:2 * NG, :].rearrange("p g c -> p (g c)"),
                             obd, attn_e.rearrange("p g c -> p (g c)"),
                             start=True, stop=True)
            rz = ap_.tile([GP, NG, 24], BF16, name=f"rz_{st}_{a}", tag="rz")
            with nc.allow_low_precision(reason="bf16 softmax"):
                nc.vector.reciprocal(
                    rz.rearrange("p g c -> p (g c)"),
                    scz[:, NG:2 * NG, :].rearrange("p g c -> p (g c)"))
                attn_n = ap_.tile([GP, NG, 24], BF16, name=f"an_{st}_{a}",
                                  tag="an")
                nc.vector.tensor_tensor(out=attn_n, in0=attn_e, in1=rz,
                                        op=ALU.mult)
            bd = bdp.tile([GP, NG, H, 48], BF16, name=f"bd_{st}_{a}", tag="bd")
            nc.gpsimd.local_scatter(
                bd.rearrange("p g h c -> p (g h c)"),
                attn_n.rearrange("p g c -> p (g c)"),
                idx[:, :], channels=GP, num_elems=NG * H * 48, num_idxs=96)
            for g in range(NG):
                vh_sb = vh_sbs[g]
                cb_ps = pp.tile([128, NC4, S, G], F32, name=f"cb_{st}_{a}_{g}",
                                tag="cb", bufs=1)
                cbf = cb_ps.rearrange("p c s n -> p c (s n)")
                for c in range(NC4):
                    nc.tensor.matmul(cbf[0:64, c, :],
                                     vh_sb[0:GP, 128 * c:128 * c + 64],
                                     bd[:, g, 2 * c, :], start=True, stop=True)
                    nc.tensor.matmul(cbf[64:128, c, :],
                                     vh_sb[0:GP, 128 * c + 64:128 * (c + 1)],
                                     bd[:, g, 2 * c + 1, :], start=True, stop=True)
                g16 = a * A_SAMP + g * G
                if g % 2 == 0:
                    nc.vector.tensor_scalar_add(oT[:, :, :, g16:g16 + G], cb_ps, 0.0)
                else:
                    nc.scalar.copy(oT[:, :, :, g16:g16 + G], cb_ps)

        def emit_B_chunks(st):
            """out-proj + LN1 + (+se) + transposes -> t_sb2, qT per slot."""
            oT = oT_tiles[st % 2]
            for s in range(S):
                t2 = t2_tiles[s]
                qT = qT_tiles[s]
                for c in range(NC4):
                    ao_ps = pp.tile([128, D], F32, name=f"ao_{st}_{s}_{c}",
                                    tag="pbig1")
                    for k in range(4):
                        nc.tensor.matmul(
                            ao_ps, oT[:, k, s, c * 128:(c + 1) * 128],
                            owT[:, k, :], start=(k == 0), stop=False)
                    nc.tensor.matmul(ao_ps, ones1, xbr[:, s, :],
                                     start=False, stop=True)
                    mv1, rstd1 = ln_rstd(ao_ps, f"1_{st}_{s}_{c}")
                    t_sb = tp.tile([128, D], BF16, name=f"t_{st}_{s}_{c}",
                                   tag="t")
                    with nc.allow_low_precision(reason="bf16 ln"):
                        nc.vector.tensor_scalar(
                            out=t_sb, in0=ao_ps,
                            scalar1=mv1[:, 0:1], scalar2=rstd1[:, 0:1],
                            op0=ALU.subtract, op1=ALU.mult)
                        if has_g1:
                            nc.vector.tensor_mul(t_sb, t_sb, g1b)
                        nc.vector.tensor_tensor(out=t2[:, c, :], in0=t_sb,
                                                in1=ser[:, s, :], op=ALU.add)
                    tr_ps = pp.tile([128, 4, 128], BF16, name=f"tr_{st}_{s}_{c}",
                                    tag="tr", bufs=1)
                    for k in range(4):
                        nc.tensor.transpose(
                            tr_ps[:, k, :], t2[:, c, 128 * k:128 * (k + 1)], idn)
                    nc.scalar.copy(qT[:, :, c * 128:(c + 1) * 128], tr_ps)

        def emit_B_ff1(st):
            for s in range(S):
                qT = qT_tiles[s]
                ff1 = ff1_tiles[s]
                for f in range(8):
                    f1_ps = pp.tile([128, D], F32, name=f"f1_{st}_{s}_{f}",
                                    tag="pbig1")
                    for k in range(4):
                        nc.tensor.matmul(f1_ps,
                                         w1T[:, k, 128 * f:128 * (f + 1)],
                                         qT[:, k, :], start=(k == 0),
                                         stop=(k == 3))
                    nc.scalar.activation(ff1[:, f, :], f1_ps, ACT_F.Gelu)

        def emit_B_ff2_slot(st, s):
            nb = st * ST_SAMP
            if True:
                t2 = t2_tiles[s]
                ff1 = ff1_tiles[s]
                y_sb = yp.tile([128, NC4, D], F32, name=f"y_{st}_{s}", tag="y")
                for c in range(NC4):
                    f2_ps = pp.tile([128, D], F32, name=f"f2_{st}_{s}_{c}",
                                    tag="pbig2")
                    nc.tensor.matmul(f2_ps, idn, t2[:, c, :],
                                     start=True, stop=False)
                    for f in range(8):
                        nc.tensor.matmul(f2_ps, ff1[:, f, c * 128:(c + 1) * 128],
                                         w2T[:, f, :], start=False,
                                         stop=(f == 7))
                    mv2, rstd2 = ln_rstd(f2_ps, f"2_{st}_{s}_{c}")
                    nc.vector.tensor_scalar(
                        out=y_sb[:, c, :], in0=f2_ps,
                        scalar1=mv2[:, 0:1], scalar2=rstd2[:, 0:1],
                        op0=ALU.subtract, op1=ALU.mult)
                    if has_g2:
                        nc.vector.tensor_mul(y_sb[:, c, :], y_sb[:, c, :], g2b)
                    if has_b2n:
                        nc.vector.tensor_add(y_sb[:, c, :], y_sb[:, c, :], b2nb)
                nc.gpsimd.dma_start(
                    out=bass.AP(tensor=out_d, offset=nb * S * D + s * D,
                                ap=[[S * D, 128], [128 * S * D, NC4], [1, D]]),
                    in_=y_sb)

        rep_ctx = tc.For_i(0, reps, 1) if reps > 1 else None
        if rep_ctx is not None:
            rep_ctx.__enter__()

        oT_tiles = [otp.tile([128, 4, S, ST_SAMP], BF16, name=f"oT_{i}",
                             tag="oT") for i in range(2)]
        t2_tiles = [tp.tile([128, NC4, D], BF16, name=f"t2_{i}", tag="t2",
                             bufs=S) for i in range(S)]
        qT_tiles = [qtp.tile([128, 4, ST_SAMP], BF16, name=f"qT_{i}", tag="qT",
                             bufs=S) for i in range(S)]
        ff1_tiles = [ff1p.tile([128, 8, ST_SAMP], BF16, name=f"ff1_{i}",
                               tag="ff1", bufs=S) for i in range(S)]

        for it in range(N_ST + 1):
            if it >= 1:
                emit_B_chunks(it - 1)
            if it < N_ST:
                for a in range(N_A):
                    emit_A(it, a)
            if it >= 1:
                emit_B_ff1(it - 1)
                for s in range(S):
                    emit_B_ff2_slot(it - 1, s)

        if rep_ctx is not None:
            rep_ctx.__exit__(None, None, None)

    nc.compile()
    return nc


def _host_prep(cand, slot_q, slot_se, in_w, in_b, out_w, out_b,
               g1, b1n, w1, b1f, w2, b2f, g2, b2n, comb_bf16=True):
    import ml_dtypes
    f32 = np.float32
    bf16 = ml_dtypes.bfloat16
    Wq, Wk, Wv = (in_w[:D], in_w[D:2 * D], in_w[2 * D:])
    bq, bk, bv = (in_b[:D], in_b[D:2 * D], in_b[2 * D:])

    qh = (slot_q @ Wq.T + bq).reshape(S, H, HD)
    Qs = np.zeros((24, D), f32)
    Wk_h = Wk.reshape(H, HD, D)
    for h in range(H):
        Qs[h * 3:(h + 1) * 3, :] = (qh[:, h, :] @ Wk_h[h]) / np.sqrt(HD)

    ob2 = out_w @ bv + out_b
    xb = (slot_q + ob2[None, :]).astype(f32)
    se = (b1n[None, :] + slot_se).astype(f32)

    # scatter indices: idx[(n,k),(g,h,s)] = g*(H*48) + h*48 + s*G + n
    idxs = np.zeros((GP, 96), np.int16)
    n_i = np.arange(G)
    for g in range(NG):
        for h in range(H):
            for s in range(S):
                idxs[:, g * 24 + h * 3 + s] = np.repeat(
                    g * H * 48 + h * 48 + s * G + n_i, K)
    obd = np.zeros((GP, GP), f32)
    for n in range(G):
        obd[n * K:(n + 1) * K, n * K:(n + 1) * K] = 1.0

    consts = {
        "qsT": np.ascontiguousarray(Qs.T).astype(bf16),
        "wvT": np.ascontiguousarray(Wv.T).astype(bf16),
        "owT": np.ascontiguousarray(out_w.T).astype(bf16),
        "w1T": np.ascontiguousarray(w1.T).astype(bf16),
        "w2T": np.ascontiguousarray(w2.T).astype(bf16),
        "xbr": xb.reshape(1, S * D).astype(bf16),
        "ser": se.reshape(1, S * D).astype(bf16),
        "one": np.ones((1, 128), bf16),
        "idx": idxs,
        "obd": obd.astype(bf16),
        "idn": np.eye(128, dtype=bf16),
    }
    flags = (not np.allclose(g1, 1.0), not np.allclose(g2, 1.0),
             not np.allclose(b2n, 0.0))
    if flags[0]:
        consts["g1v"] = g1.astype(f32)
    if flags[1]:
        consts["g2v"] = g2.astype(f32)
    if flags[2]:
        consts["b2nv"] = b2n.astype(f32)

    kvT = np.ascontiguousarray(
        cand.reshape(B, T * K, D).transpose(0, 2, 1)).astype(bf16)
    return kvT, consts, flags


COMB_BF16 = True


def kernel(**inputs):
    kvT, consts, flags = _host_prep(**inputs, comb_bf16=COMB_BF16)
    key = flags + (COMB_BF16,)
    if key not in _CACHE:
        _CACHE[key] = _build(*flags, comb_bf16=COMB_BF16)
    nc = _CACHE[key]
    in_maps = [dict(consts, kvT=kvT[c]) for c in range(NCORES)]
    res = run_bass_kernel_spmd(nc, in_maps, list(range(NCORES)))
    out = np.concatenate([res.results[c]["out"] for c in range(NCORES)], axis=0)
    return out.astype(np.float32)


if __name__ == "__main__":
    import reference
    import jax as _jax
    with _jax.default_device(_jax.devices("cpu")[0]):
        ins = {k: np.asarray(v) for k, v in reference.setup_inputs().items()}
        exp = np.asarray(reference.reference(**ins))
    got = kernel(**ins)
    rel = np.sqrt(((got - exp) ** 2).mean() / ((exp ** 2).mean() + 1e-30))
    print("shape", got.shape, "rms rel err:", rel)


# revision 4
# speedup vs baseline: 1.0484x; 1.0484x over previous
"""CrossAttentionQuerySelector TRN2 kernel v2 (8-core data parallel).

All-bf16 matmul design. Per core (2048 samples, K=7, D=512, H=8, S=3):

Phase A (per 64-sample tile, 4 groups of 16 samples = 112 (n,k)-partitions):
  - kvT [512, 14336] bf16 loaded [128,4,448]; scores computed DIRECTLY in
    (n,k)-partition layout: sc[(n,k),(h,s)] = kvT_g.T @ QsT  (Wk folded into
    the 24 slot queries on host).
  - softmax over k (on partitions): exp on ACT; Z via block-ones matmul
    (ones_bd.T @ exp broadcasts group sums back to all 112 partitions);
    reciprocal + multiply on DVE (bf16 2x).
  - attn diagonalized into a block-diagonal [112, 4*8*48] operand with ONE
    gpsimd local_scatter (per-partition indices, auto-zeroed dest).
  - vh' [(n,k), e] = kvT_g.T @ WvT (4 accumulating mms per group).
  - combine ON PE: per (group, e-chunk) two 48-row matmuls
    oT[e, (s,n)] = vh'.T @ bd_head, written at psum partition offsets 0/64.
Phase B (per slot, 512-sample block, pipelined one supertile behind A):
  - out-proj (+xbr bias via ones-matmul), LN1 stats on DVE, rstd=exp(-ln/2)
    on ACT (shared table set with softmax exp), normalize on DVE,
    q_se = t + (b1n+slot_se) on DVE (bf16 2x), PE transposes -> qT,
  - FFN: ff1+gelu; ff2 accumulates on top of an identity-matmul of q_se
    (residual folded into psum), LN2, store.
Emission order per supertile: B.chunks(st-1), A(st), B.ff1(st-1),
B.ff2(st-1) -- keeps ACT table reloads to 2/supertile and overlaps the
DVE/ACT-heavy A phase with the PE-heavy FFN.
"""

import os
import sys

for _p in ("/opt/trn_rl_repo", "/root/.axon_site/_ro/trn_rl_repo"):
    if os.path.isdir(_p) and _p not in sys.path:
        sys.path.insert(0, _p)

import numpy as np
from contextlib import ExitStack

import concourse.bass as bass
import concourse.tile as tile
from concourse import mybir, bacc
from concourse.bass_utils import run_bass_kernel_spmd

# Steer Exp/Ln onto the combined `natural_log_exp_and_others` table set so
# softmax-exp and the LN rstd (exp(-ln/2)) share one set: hide Exp/Ln in
# every other set handed to the load-insertion pass.
_orig_get_tables = bacc.get_activation_tables


def _patched_get_tables(arch):
    tabs = _orig_get_tables(arch)
    out = {}
    for name, funcs in tabs.items():
        if name != "natural_log_exp_and_others":
            funcs = funcs - {mybir.ActivationFunctionType.Exp,
                             mybir.ActivationFunctionType.Ln}
        out[name] = funcs
    return out


bacc.get_activation_tables = _patched_get_tables

F32 = mybir.dt.float32
BF16 = mybir.dt.bfloat16
I16 = mybir.dt.int16
AX = mybir.AxisListType
ALU = mybir.AluOpType
ACT_F = mybir.ActivationFunctionType

D = 512
H = 8
HD = 64
S = 3
K = 7
B = 8
T = 2048
NCORES = 8
NSAMP = T
ROWS = NSAMP * K               # 14336
ST_SAMP = 512
N_ST = NSAMP // ST_SAMP        # 4
A_SAMP = 64                    # samples per phase-A tile
A_ROWS = A_SAMP * K            # 448
N_A = ST_SAMP // A_SAMP        # 8
G = 16                         # samples per combine group
GP = G * K                     # 112 partitions per group
NG = A_SAMP // G               # 4 groups per A-tile
NC4 = ST_SAMP // 128           # 4 sample chunks per phase-B block
EPS = 1e-5

_CACHE = {}


def _build(has_g1, has_g2, has_b2n, reps=1, comb_bf16=True):
    nc = bacc.Bacc("TRN2", target_bir_lowering=False, debug=False,
                   num_devices=NCORES)

    kvT_d = nc.dram_tensor("kvT", [D, ROWS], BF16, kind="ExternalInput")
    qsT_d = nc.dram_tensor("qsT", [D, 24], BF16, kind="ExternalInput")
    wvT_d = nc.dram_tensor("wvT", [D, D], BF16, kind="ExternalInput")
    owT_d = nc.dram_tensor("owT", [D, D], BF16, kind="ExternalInput")
    w1T_d = nc.dram_tensor("w1T", [D, 2 * D], BF16, kind="ExternalInput")
    w2T_d = nc.dram_tensor("w2T", [2 * D, D], BF16, kind="ExternalInput")
    xbr_d = nc.dram_tensor("xbr", [1, S * D], BF16, kind="ExternalInput")
    ser_d = nc.dram_tensor("ser", [1, S * D], BF16, kind="ExternalInput")
    one_d = nc.dram_tensor("one", [1, 128], BF16, kind="ExternalInput")
    idx_d = nc.dram_tensor("idx", [GP, 96], I16, kind="ExternalInput")
    obd_d = nc.dram_tensor("obd", [GP, GP], BF16, kind="ExternalInput")
    idn_d = nc.dram_tensor("idn", [128, 128], BF16, kind="ExternalInput")
    g1_d = g2_d = b2n_d = None
    if has_g1:
        g1_d = nc.dram_tensor("g1v", [D], F32, kind="ExternalInput")
    if has_g2:
        g2_d = nc.dram_tensor("g2v", [D], F32, kind="ExternalInput")
    if has_b2n:
        b2n_d = nc.dram_tensor("b2nv", [D], F32, kind="ExternalInput")
    out_d = nc.dram_tensor("out", [NSAMP, S, D], F32, kind="ExternalOutput")

    with tile.TileContext(nc) as tc, ExitStack() as ctx:
        cp = ctx.enter_context(tc.tile_pool(name="consts", bufs=1))
        kvp = ctx.enter_context(tc.tile_pool(name="kvp", bufs=3))
        ap_ = ctx.enter_context(tc.tile_pool(name="aphase", bufs=2))
        bdp = ctx.enter_context(tc.tile_pool(name="bdp", bufs=2))
        vsp = ctx.enter_context(tc.tile_pool(name="vsp", bufs=2))
        otp = ctx.enter_context(tc.tile_pool(name="otp", bufs=2))
        tp = ctx.enter_context(tc.tile_pool(name="tp", bufs=2))
        qtp = ctx.enter_context(tc.tile_pool(name="qtp", bufs=2))
        ff1p = ctx.enter_context(tc.tile_pool(name="ff1p", bufs=2))
        yp = ctx.enter_context(tc.tile_pool(name="yp", bufs=2))
        sp = ctx.enter_context(tc.tile_pool(name="smalls", bufs=6))
        pp = ctx.enter_context(tc.tile_pool(name="psum", bufs=2, space="PSUM"))

        # ---- constants ----
        qsT = cp.tile([128, 4, 24], BF16, name="qsT")
        wvT = cp.tile([128, 4, D], BF16, name="wvT")
        owT = cp.tile([128, 4, D], BF16, name="owT")
        w1T = cp.tile([128, 4, 2 * D], BF16, name="w1T")
        w2T = cp.tile([128, 8, D], BF16, name="w2T")
        for k in range(4):
            nc.scalar.dma_start(out=qsT[:, k, :], in_=qsT_d[128 * k:128 * (k + 1), :])
            nc.scalar.dma_start(out=wvT[:, k, :], in_=wvT_d[128 * k:128 * (k + 1), :])
            nc.scalar.dma_start(out=owT[:, k, :], in_=owT_d[128 * k:128 * (k + 1), :])
            nc.scalar.dma_start(out=w1T[:, k, :], in_=w1T_d[128 * k:128 * (k + 1), :])
        for k in range(8):
            nc.scalar.dma_start(out=w2T[:, k, :], in_=w2T_d[128 * k:128 * (k + 1), :])
        xbr = cp.tile([1, S, D], BF16, name="xbr")
        nc.scalar.dma_start(out=xbr, in_=xbr_d.ap().rearrange("p (s n) -> p s n", s=S))
        ser = cp.tile([128, S, D], BF16, name="ser")
        nc.scalar.dma_start(out=ser, in_=bass.AP(
            tensor=ser_d, offset=0,
            ap=[[0, 128], [D, S], [1, D]]))
        ones1 = cp.tile([1, 128], BF16, name="ones1")
        nc.scalar.dma_start(out=ones1, in_=one_d[:])
        idx = cp.tile([GP, 96], I16, name="idx")
        nc.scalar.dma_start(out=idx, in_=idx_d[:, :])
        obd = cp.tile([GP, GP], BF16, name="obd")
        nc.scalar.dma_start(out=obd, in_=obd_d[:, :])
        idn = cp.tile([128, 128], BF16, name="idn")
        nc.scalar.dma_start(out=idn, in_=idn_d[:, :])
        epsb = cp.tile([128, 1], F32, name="epsb")
        nc.vector.memset(epsb, EPS)
        g1b = g2b = b2nb = None
        if has_g1:
            g1b = cp.tile([128, D], F32, name="g1b")
            nc.scalar.dma_start(out=g1b, in_=bass.AP(
                tensor=g1_d, offset=0, ap=[[0, 128], [1, D]]))
        if has_g2:
            g2b = cp.tile([128, D], F32, name="g2b")
            nc.scalar.dma_start(out=g2b, in_=bass.AP(
                tensor=g2_d, offset=0, ap=[[0, 128], [1, D]]))
        if has_b2n:
            b2nb = cp.tile([128, D], F32, name="b2nb")
            nc.scalar.dma_start(out=b2nb, in_=bass.AP(
                tensor=b2n_d, offset=0, ap=[[0, 128], [1, D]]))

        def ln_rstd(x_ap, tag):
            """LN stats of x_ap [128,512] -> (mv [128,2], rstd [128,1])."""
            st6 = sp.tile([128, 6], F32, name=f"st6_{tag}", tag="st6")
            nc.vector.bn_stats(out=st6, in_=x_ap)
            mv = sp.tile([128, 2], F32, name=f"mv_{tag}", tag="mv")
            nc.vector.bn_aggr(out=mv, in_=st6)
            lnv = sp.tile([128, 1], F32, name=f"lnv_{tag}", tag="lnv")
            nc.scalar.activation(lnv, mv[:, 1:2], ACT_F.Ln, bias=epsb[:, 0:1])
            rstd = sp.tile([128, 1], F32, name=f"rstd_{tag}", tag="rstd")
            nc.scalar.activation(rstd, lnv, ACT_F.Exp, scale=-0.5)
            return mv, rstd

        # ---------------- emission helpers ----------------
        def emit_A(st, a):
            """64 samples: scores/softmax/scatter/vh/combine -> oT."""
            oT = oT_tiles[st % 2]
            r0 = (st * ST_SAMP + a * A_SAMP) * K
            kv = kvp.tile([128, 4, A_ROWS], BF16, name=f"kv_{st}_{a}", tag="kv")
            nc.sync.dma_start(
                out=kv,
                in_=bass.AP(tensor=kvT_d, offset=r0,
                            ap=[[ROWS, 128], [128 * ROWS, 4], [1, A_ROWS]]))
            scz = pp.tile([GP, 2 * NG, 24], F32, name=f"scz_{st}_{a}",
                          tag="scz")
            for g in range(NG):
                for k in range(4):
                    nc.tensor.matmul(scz[:, g, :], kv[:, k, GP * g:GP * (g + 1)],
                                     qsT[:, k, :], start=(k == 0), stop=(k == 3))
            attn_e = ap_.tile([GP, NG, 24], BF16, name=f"ae_{st}_{a}", tag="ae")
            nc.scalar.activation(attn_e, scz[:, 0:NG, :], ACT_F.Exp)
            nc.tensor.matmul(scz[:, NG:2 * NG, :].rearrange("p g c -> p (g c)"),
                             obd, attn_e.rearrange("p g c -> p (g c)"),
                             start=True, stop=True)
            rz = ap_.tile([GP, NG, 24], BF16, name=f"rz_{st}_{a}", tag="rz")
            with nc.allow_low_precision(reason="bf16 softmax"):
                nc.vector.reciprocal(
                    rz.rearrange("p g c -> p (g c)"),
                    scz[:, NG:2 * NG, :].rearrange("p g c -> p (g c)"))
                attn_n = ap_.tile([GP, NG, 24], BF16, name=f"an_{st}_{a}",
                                  tag="an")
                nc.vector.tensor_tensor(out=attn_n, in0=attn_e, in1=rz,
                                        op=ALU.mult)
            bd = bdp.tile([GP, NG, H, 48], BF16, name=f"bd_{st}_{a}", tag="bd")
            nc.gpsimd.local_scatter(
                bd.rearrange("p g h c -> p (g h c)"),
                attn_n.rearrange("p g c -> p (g c)"),
                idx[:, :], channels=GP, num_elems=NG * H * 48, num_idxs=96)
            for g in range(NG):
                vh_ps = pp.tile([128, D], F32, name=f"vh_{st}_{a}_{g}",
                                tag="pbig2")
                for k in range(4):
                    nc.tensor.matmul(vh_ps[0:GP, :], kv[:, k, GP * g:GP * (g + 1)],
                                     wvT[:, k, :], start=(k == 0), stop=(k == 3))
                vh_sb = vsp.tile([128, D], BF16, name=f"vs_{st}_{a}_{g}",
                                 tag="vs")
                if g % 2 == 0:
                    nc.scalar.copy(vh_sb[0:GP, :], vh_ps[0:GP, :])
                else:
                    nc.vector.tensor_scalar_add(vh_sb[0:GP, :], vh_ps[0:GP, :], 0.0)
                cb_ps = pp.tile([128, NC4, S, G], F32, name=f"cb_{st}_{a}_{g}",
                                tag="cb", bufs=1)
                cbf = cb_ps.rearrange("p c s n -> p c (s n)")
                for c in range(NC4):
                    nc.tensor.matmul(cbf[0:64, c, :],
                                     vh_sb[0:GP, 128 * c:128 * c + 64],
                                     bd[:, g, 2 * c, :], start=True, stop=True)
                    nc.tensor.matmul(cbf[64:128, c, :],
                                     vh_sb[0:GP, 128 * c + 64:128 * (c + 1)],
                                     bd[:, g, 2 * c + 1, :], start=True, stop=True)
                g16 = a * A_SAMP + g * G
                if g % 2 == 0:
                    nc.vector.tensor_scalar_add(oT[:, :, :, g16:g16 + G], cb_ps, 0.0)
                else:
                    nc.scalar.copy(oT[:, :, :, g16:g16 + G], cb_ps)

        def emit_B_chunks(st):
            """out-proj + LN1 + (+se) + transposes -> t_sb2, qT per slot."""
            oT = oT_tiles[st % 2]
            for s in range(S):
                t2 = t2_tiles[s]
                qT = qT_tiles[s]
                for c in range(NC4):
                    ao_ps = pp.tile([128, D], F32, name=f"ao_{st}_{s}_{c}",
                                    tag="pbig1")
                    for k in range(4):
                        nc.tensor.matmul(
                            ao_ps, oT[:, k, s, c * 128:(c + 1) * 128],
                            owT[:, k, :], start=(k == 0), stop=False)
                    nc.tensor.matmul(ao_ps, ones1, xbr[:, s, :],
                                     start=False, stop=True)
                    mv1, rstd1 = ln_rstd(ao_ps, f"1_{st}_{s}_{c}")
                    t_sb = tp.tile([128, D], BF16, name=f"t_{st}_{s}_{c}",
                                   tag="t")
                    with nc.allow_low_precision(reason="bf16 ln"):
                        nc.vector.tensor_scalar(
                            out=t_sb, in0=ao_ps,
                            scalar1=mv1[:, 0:1], scalar2=rstd1[:, 0:1],
                            op0=ALU.subtract, op1=ALU.mult)
                        if has_g1:
                            nc.vector.tensor_mul(t_sb, t_sb, g1b)
                        nc.vector.tensor_tensor(out=t2[:, c, :], in0=t_sb,
                                                in1=ser[:, s, :], op=ALU.add)
                    tr_ps = pp.tile([128, 4, 128], BF16, name=f"tr_{st}_{s}_{c}",
                                    tag="tr", bufs=1)
                    for k in range(4):
                        nc.tensor.transpose(
                            tr_ps[:, k, :], t2[:, c, 128 * k:128 * (k + 1)], idn)
                    nc.scalar.copy(qT[:, :, c * 128:(c + 1) * 128], tr_ps)

        def emit_B_ff1(st):
            for s in range(S):
                qT = qT_tiles[s]
                ff1 = ff1_tiles[s]
                for f in range(8):
                    f1_ps = pp.tile([128, D], F32, name=f"f1_{st}_{s}_{f}",
                                    tag="pbig1")
                    for k in range(4):
                        nc.tensor.matmul(f1_ps,
                                         w1T[:, k, 128 * f:128 * (f + 1)],
                                         qT[:, k, :], start=(k == 0),
                                         stop=(k == 3))
                    nc.scalar.activation(ff1[:, f, :], f1_ps, ACT_F.Gelu)

        def emit_B_ff2_slot(st, s):
            nb = st * ST_SAMP
            if True:
                t2 = t2_tiles[s]
                ff1 = ff1_tiles[s]
                y_sb = yp.tile([128, NC4, D], F32, name=f"y_{st}_{s}", tag="y")
                for c in range(NC4):
                    f2_ps = pp.tile([128, D], F32, name=f"f2_{st}_{s}_{c}",
                                    tag="pbig2")
                    nc.tensor.matmul(f2_ps, idn, t2[:, c, :],
                                     start=True, stop=False)
                    for f in range(8):
                        nc.tensor.matmul(f2_ps, ff1[:, f, c * 128:(c + 1) * 128],
                                         w2T[:, f, :], start=False,
                                         stop=(f == 7))
                    mv2, rstd2 = ln_rstd(f2_ps, f"2_{st}_{s}_{c}")
                    nc.vector.tensor_scalar(
                        out=y_sb[:, c, :], in0=f2_ps,
                        scalar1=mv2[:, 0:1], scalar2=rstd2[:, 0:1],
                        op0=ALU.subtract, op1=ALU.mult)
                    if has_g2:
                        nc.vector.tensor_mul(y_sb[:, c, :], y_sb[:, c, :], g2b)
                    if has_b2n:
                        nc.vector.tensor_add(y_sb[:, c, :], y_sb[:, c, :], b2nb)
                nc.sync.dma_start(
                    out=bass.AP(tensor=out_d, offset=nb * S * D + s * D,
                                ap=[[S * D, 128], [128 * S * D, NC4], [1, D]]),
                    in_=y_sb)

        rep_ctx = tc.For_i(0, reps, 1) if reps > 1 else None
        if rep_ctx is not None:
            rep_ctx.__enter__()

        oT_tiles = [otp.tile([128, 4, S, ST_SAMP], BF16, name=f"oT_{i}",
                             tag="oT") for i in range(2)]
        t2_tiles = [tp.tile([128, NC4, D], BF16, name=f"t2_{i}", tag="t2",
                             bufs=S) for i in range(S)]
        qT_tiles = [qtp.tile([128, 4, ST_SAMP], BF16, name=f"qT_{i}", tag="qT",
                             bufs=S) for i in range(S)]
        ff1_tiles = [ff1p.tile([128, 8, ST_SAMP], BF16, name=f"ff1_{i}",
                               tag="ff1", bufs=S) for i in range(S)]

        for it in range(N_ST + 1):
            if it >= 1:
                emit_B_chunks(it - 1)
            if it < N_ST:
                for a in range(N_A):
                    emit_A(it, a)
            if it >= 1:
                emit_B_ff1(it - 1)
                for s in range(S):
                    emit_B_ff2_slot(it - 1, s)

        if rep_ctx is not None:
            rep_ctx.__exit__(None, None, None)

    nc.compile()
    return nc


def _host_prep(cand, slot_q, slot_se, in_w, in_b, out_w, out_b,
               g1, b1n, w1, b1f, w2, b2f, g2, b2n, comb_bf16=True):
    import ml_dtypes
    f32 = np.float32
    bf16 = ml_dtypes.bfloat16
    Wq, Wk, Wv = (in_w[:D], in_w[D:2 * D], in_w[2 * D:])
    bq, bk, bv = (in_b[:D], in_b[D:2 * D], in_b[2 * D:])

    qh = (slot_q @ Wq.T + bq).reshape(S, H, HD)
    Qs = np.zeros((24, D), f32)
    Wk_h = Wk.reshape(H, HD, D)
    for h in range(H):
        Qs[h * 3:(h + 1) * 3, :] = (qh[:, h, :] @ Wk_h[h]) / np.sqrt(HD)

    ob2 = out_w @ bv + out_b
    xb = (slot_q + ob2[None, :]).astype(f32)
    se = (b1n[None, :] + slot_se).astype(f32)

    # scatter indices: idx[(n,k),(g,h,s)] = g*(H*48) + h*48 + s*G + n
    idxs = np.zeros((GP, 96), np.int16)
    n_i = np.arange(G)
    for g in range(NG):
        for h in range(H):
            for s in range(S):
                idxs[:, g * 24 + h * 3 + s] = np.repeat(
                    g * H * 48 + h * 48 + s * G + n_i, K)
    obd = np.zeros((GP, GP), f32)
    for n in range(G):
        obd[n * K:(n + 1) * K, n * K:(n + 1) * K] = 1.0

    consts = {
        "qsT": np.ascontiguousarray(Qs.T).astype(bf16),
        "wvT": np.ascontiguousarray(Wv.T).astype(bf16),
        "owT": np.ascontiguousarray(out_w.T).astype(bf16),
        "w1T": np.ascontiguousarray(w1.T).astype(bf16),
        "w2T": np.ascontiguousarray(w2.T).astype(bf16),
        "xbr": xb.reshape(1, S * D).astype(bf16),
        "ser": se.reshape(1, S * D).astype(bf16),
        "one": np.ones((1, 128), bf16),
        "idx": idxs,
        "obd": obd.astype(bf16),
        "idn": np.eye(128, dtype=bf16),
    }
    flags = (not np.allclose(g1, 1.0), not np.allclose(g2, 1.0),
             not np.allclose(b2n, 0.0))
    if flags[0]:
        consts["g1v"] = g1.astype(f32)
    if flags[1]:
        consts["g2v"] = g2.astype(f32)
    if flags[2]:
        consts["b2nv"] = b2n.astype(f32)

    kvT = np.ascontiguousarray(
        cand.reshape(B, T * K, D).transpose(0, 2, 1)).astype(bf16)
    return kvT, consts, flags


COMB_BF16 = True


def kernel(**inputs):
    kvT, consts, flags = _host_prep(**inputs, comb_bf16=COMB_BF16)
    key = flags + (COMB_BF16,)
    if key not in _CACHE:
        _CACHE[key] = _build(*flags, comb_bf16=COMB_BF16)
    nc = _CACHE[key]
    in_maps = [dict(consts, kvT=kvT[c]) for c in range(NCORES)]
    res = run_bass_kernel_spmd(nc, in_maps, list(range(NCORES)))
    out = np.concatenate([res.results[c]["out"] for c in range(NCORES)], axis=0)
    return out.astype(np.float32)


if __name__ == "__main__":
    import reference
    import jax as _jax
    with _jax.default_device(_jax.devices("cpu")[0]):
        ins = {k: np.asarray(v) for k, v in reference.setup_inputs().items()}
        exp = np.asarray(reference.reference(**ins))
    got = kernel(**ins)
    rel = np.sqrt(((got - exp) ** 2).mean() / ((exp ** 2).mean() + 1e-30))
    print("shape", got.shape, "rms rel err:", rel)
